# revision 4
# baseline (speedup 1.0000x reference)
"""Causal single-head attention on 8 Trainium2 NeuronCores (v2).

Problem: x[4, 4096, 1024], Wq/Wk/Wv[1024, 64] ->
  out = softmax(causal(Q K^T / 8)) V   per batch, fp32.

Sharding: core i handles batch b = i//2 with query parity p = i%2. Each core
owns a 256-row half of every 512-wide query chunk (p=0 the first half, p=1 the
second). Both cores of a pair load the full x[b] (transposed to [C, T] and
bf16-cast on host) and compute full K/V.

The SPMD program is identical on all cores; parity enters only through data:
  - x columns are permuted per core so that within each 512 chunk the OTHER
    core's 256 queries come first and the OWNED 256 queries second. Owned
    queries therefore sit at static addresses on every core.
  - a per-core score mask [128, 1024] covering key tiles 4g..4g+3 of slot g:
    the first half kills the "other half" tiles on p=0 cores (those keys are
    in the causal future there), the second half is the triangular diagonal
    mask (identical on both cores).

Per chunk g, slot g processes the owned 256 queries against key tiles
0..4g+3 (128 keys each): tiles < 4g are strictly causal-past (no mask),
tiles 4g..4g+3 get the mask add (one DVE op per slot).

Scores are computed transposed (S^T[k, q]) so softmax'd weights feed the PV
matmul directly with k on partitions; V is augmented with a ones column so row
sums accumulate alongside P@V. exp() batches 4 score tiles (2 PSUM banks) per
ACT call. The unnormalized [65, 256] accumulator (P@V rows 0:64, row sums in
row 64) is DMA'd straight from PSUM to DRAM; the final divide + transpose to
[q, h] happens on the host in fp64.

All matmul operands are bf16 (measured kernel-vs-fp64 rel err ~4e-3, budget
2e-2); accumulation is fp32 in PSUM.
"""

import numpy as np
import ml_dtypes

import concourse.bacc as bacc
import concourse.mybir as mybir
import concourse.tile as tile
from concourse.bass_utils import run_bass_kernel_spmd

# Problem dims
B, T, C, HS = 4, 4096, 1024, 64
P = 128            # partitions
CH = 512           # x chunk width
NCH = T // CH      # 8 chunks == 8 slots
QW = 256           # owned query width per slot
CSUB = C // P      # 8 contraction subtiles
NKT = T // P       # 32 key tiles
NEG = -1.0e9

IN_DT = mybir.dt.bfloat16
SCALE = float(HS) ** -0.5


def _build_program():
    nc = bacc.Bacc("TRN2")
    f32 = mybir.dt.float32
    EXP = mybir.ActivationFunctionType.Exp

    xT = nc.dram_tensor("xT", [C, T], IN_DT, kind="ExternalInput").ap()
    wqk = nc.dram_tensor("wqk", [C, P], IN_DT, kind="ExternalInput").ap()
    wv = nc.dram_tensor("wv", [C, HS], IN_DT, kind="ExternalInput").ap()
    mask_d = nc.dram_tensor("mask", [P, 2 * QW], f32, kind="ExternalInput").ap()
    bflag_d = nc.dram_tensor("bflag", [P, 1], f32, kind="ExternalInput").ap()
    out_d = nc.dram_tensor("out", [NCH, HS + 1, QW], f32, kind="ExternalOutput").ap()

    xT_r = xT.rearrange("(co ci) t -> ci co t", ci=P)     # [128, 8, 4096]
    wqk_r = wqk.rearrange("(co ci) m -> ci co m", ci=P)   # [128, 8, 128]
    wv_r = wv.rearrange("(co ci) m -> ci co m", ci=P)     # [128, 8, 64]

    with tile.TileContext(nc) as tc:
        with (
            tc.tile_pool(name="const", bufs=1) as const_pool,
            tc.tile_pool(name="persist", bufs=1) as persist,
            tc.tile_pool(name="xin", bufs=3) as xpool,
            tc.tile_pool(name="vt", bufs=2) as vt_pool,
            tc.tile_pool(name="pt", bufs=4) as pt_pool,
            tc.tile_pool(name="osb", bufs=2) as osb_pool,
            tc.tile_pool(name="proj_ps", bufs=2, space="PSUM") as proj_ps,
            tc.tile_pool(name="st_ps", bufs=2, space="PSUM") as st_ps,
            tc.tile_pool(name="stm_ps", bufs=1, space="PSUM") as stm_ps,
            tc.tile_pool(name="ot_ps", bufs=1, space="PSUM") as ot_ps,
        ):
            # ---- constants (issued on ACT queue so x streaming owns SP) ----
            wqk_sb = const_pool.tile([P, CSUB, P], IN_DT)
            wv_sb = const_pool.tile([P, CSUB, HS], IN_DT)
            mask_sb = const_pool.tile([P, 2 * QW], f32)
            bflag_sb = const_pool.tile([P, 1], f32)
            nc.scalar.dma_start(wqk_sb[:], wqk_r)
            nc.scalar.dma_start(wv_sb[:], wv_r)
            nc.scalar.dma_start(mask_sb[:], mask_d)
            nc.scalar.dma_start(bflag_sb[:], bflag_d)

            kt_all = persist.tile([HS, T], IN_DT)           # K^T
            qt_all = persist.tile([HS, NCH, QW], IN_DT)     # owned Q^T halves
            # inner dim padded 65->80: XBAR dma-transpose destinations must
            # be 32B-aligned (HW-probed; 144B strides corrupt, 160B work)
            v_all = persist.tile([P, NKT, 80], IN_DT)       # V with ones column
            nc.gpsimd.memset(v_all[:, :, HS : HS + 1], 1.0)

            # Warm the PE p-state ramp and the ACT exp table before real work.
            scratch = const_pool.tile([HS, CH], IN_DT)
            warm_sb = const_pool.tile([P, 1], f32)
            nc.gpsimd.memset(scratch[:], 0.0)
            nc.gpsimd.memset(warm_sb[:], 0.0)
            warm_ps = ot_ps.tile([P, QW], f32, tag="ot")
            for _ in range(12):
                nc.tensor.matmul(
                    warm_ps[:], lhsT=scratch[:, 0:P], rhs=scratch[:, 0:QW],
                    start=True, stop=True,
                )
            warm_pt = const_pool.tile([P, 1], f32)
            nc.scalar.activation(warm_pt[:], warm_sb[:], EXP, scale=0.0)

            def s_batch(g, m, q_ap):
                """Score 4 unmasked key tiles into 2 PSUM banks, exp to pt."""
                st = st_ps.tile([P, 2, 2 * QW], f32, tag="st")
                for i in range(4):
                    nc.tensor.matmul(
                        st[:, i // 2, (i % 2) * QW : (i % 2 + 1) * QW],
                        lhsT=kt_all[:, (4 * m + i) * P : (4 * m + i + 1) * P],
                        rhs=q_ap,
                        start=True, stop=True,
                    )
                pt = pt_pool.tile([P, 4 * QW], IN_DT, tag="pt")
                nc.scalar.activation(pt[:], st[:], EXP, scale=SCALE)
                return pt

            def masked_batch(g, q_ap):
                """Key tiles 4g..4g+3: other-half pair (activation-bias
                mask, no DVE op) then diagonal pair (triangular DVE add).
                Uses its own 1-bank pool so the long mask+exp latency never
                blocks the below-batch score buffers."""
                pt = pt_pool.tile([P, 4 * QW], IN_DT, tag="pt")
                sto = stm_ps.tile([P, 2 * QW], f32, tag="stm")
                for i in range(2):
                    nc.tensor.matmul(
                        sto[:, i * QW : (i + 1) * QW],
                        lhsT=kt_all[:, (4 * g + i) * P : (4 * g + i + 1) * P],
                        rhs=q_ap, start=True, stop=True,
                    )
                nc.scalar.activation(
                    pt[:, 0 : 2 * QW], sto[:], EXP, scale=SCALE,
                    bias=bflag_sb[:],
                )
                std = stm_ps.tile([P, 2 * QW], f32, tag="stm")
                for i in range(2):
                    nc.tensor.matmul(
                        std[:, i * QW : (i + 1) * QW],
                        lhsT=kt_all[:, (4 * g + 2 + i) * P : (4 * g + 3 + i) * P],
                        rhs=q_ap, start=True, stop=True,
                    )
                nc.vector.tensor_add(std[:], std[:], mask_sb[:])
                nc.scalar.activation(
                    pt[:, 2 * QW : 4 * QW], std[:], EXP, scale=SCALE,
                )
                return pt

            def attention(g, masked_first=False):
                """Slot g: owned queries of chunk g vs key tiles 0..4g+3.

                masked_first issues the masked group's S/exp before the
                below-batches (PV still accumulates last) so its latency
                hides behind them — used when no projection work follows."""
                q_ap = qt_all[:, g, :]
                ot = ot_ps.tile([HS + 1, QW], f32, tag="ot")

                def pv(pt, i0, first, last):
                    for i in range(4):
                        nc.tensor.matmul(
                            ot[:],
                            lhsT=v_all[:, i0 + i, 0 : HS + 1],
                            rhs=pt[:, i * QW : (i + 1) * QW],
                            start=(first and i == 0),
                            stop=(last and i == 3),
                        )

                pt_mixed = masked_batch(g, q_ap) if masked_first else None
                for m in range(g):
                    pt = s_batch(g, m, q_ap)
                    pv(pt, 4 * m, first=(m == 0), last=False)
                if pt_mixed is None:
                    pt_mixed = masked_batch(g, q_ap)
                pv(pt_mixed, 4 * g, first=(g == 0), last=True)
                # unnormalized out + row sums to DRAM; host divides
                o_sb = osb_pool.tile([HS + 1, QW], f32, tag="osb")
                nc.vector.tensor_copy(o_sb[:], ot[:])
                nc.sync.dma_start(out_d[g], o_sb[:])

            # ---- streamed projection + pipelined attention ----
            for g in range(NCH):
                xc_lo = xpool.tile([P, CSUB // 2, CH], IN_DT, tag="xlo")
                xc_hi = xpool.tile([P, CSUB // 2, CH], IN_DT, tag="xhi")
                nc.sync.dma_start(
                    xc_lo[:], xT_r[:, 0 : CSUB // 2, g * CH : (g + 1) * CH]
                )
                nc.sync.dma_start(
                    xc_hi[:], xT_r[:, CSUB // 2 : CSUB, g * CH : (g + 1) * CH]
                )
                xc_h = [xc_lo, xc_hi]

                # Q^T (rows 0:64) and K^T (rows 64:128), stacked projection
                qk_ps = proj_ps.tile([P, CH], f32, tag="proj")
                for cs in range(CSUB):
                    nc.tensor.matmul(
                        qk_ps[:],
                        lhsT=wqk_sb[:, cs, :],
                        rhs=xc_h[cs // 4][:, cs % 4, :],
                        start=(cs == 0),
                        stop=(cs == CSUB - 1),
                    )
                nc.vector.tensor_copy(qt_all[:, g, :], qk_ps[0:HS, QW:CH])
                nc.vector.tensor_copy(kt_all[:, g * CH : (g + 1) * CH], qk_ps[HS:P, :])

                # V^T projection (transposed to natural [t, h] after attention)
                vt_ps = proj_ps.tile([HS, CH], f32, tag="proj")
                for cs in range(CSUB):
                    nc.tensor.matmul(
                        vt_ps[:],
                        lhsT=wv_sb[:, cs, :],
                        rhs=xc_h[cs // 4][:, cs % 4, :],
                        start=(cs == 0),
                        stop=(cs == CSUB - 1),
                    )
                vt_sb = vt_pool.tile([HS, CH], IN_DT, tag="vt")
                nc.vector.tensor_copy(vt_sb[:], vt_ps[:])
                # V natural tiles via XBAR DMA transpose (bf16, SBUF->SBUF)
                for tt in range(CH // P):
                    nc.sync.dma_start_transpose(
                        v_all[:, 4 * g + tt, 0:HS],
                        vt_sb[:, tt * P : (tt + 1) * P],
                    )

                # attention for THIS slot: its qt/kt copies complete while
                # the V^T matmuls above keep PE busy, so no lag needed.
                attention(g, masked_first=True)

    nc.compile()
    return nc


_CACHE = {}


def _get_program():
    if "nc" not in _CACHE:
        _CACHE["nc"] = _build_program()
    return _CACHE["nc"]


def _host_inputs(x, Wk, Wq, Wv):
    bf16 = ml_dtypes.bfloat16
    x = np.asarray(x, dtype=np.float32)
    wqk = np.ascontiguousarray(
        np.concatenate([np.asarray(Wq), np.asarray(Wk)], axis=1), dtype=bf16
    )
    wv = np.ascontiguousarray(np.asarray(Wv), dtype=bf16)

    # xT per core: [C, T] with each 512 chunk's columns as [other 256 | own 256]
    # p=0 owns the first 256 of each chunk -> order [1, 0]; p=1 -> [0, 1].
    xT = []
    for b in range(B):
        xt = np.ascontiguousarray(x[b].T).reshape(C, NCH, 2, QW)
        xT.append([
            np.ascontiguousarray(
                xt[:, :, [1 - p, p], :].reshape(C, T), dtype=bf16
            )
            for p in (0, 1)
        ])

    # per-core mask over key tiles 4g..4g+3 of slot g: [128, 1024] where
    # cols 0:512 cover the other-half pair (all-masked on p=0, open on p=1)
    # and cols 512:1024 the triangular diagonal pair (same on both cores).
    kk = np.arange(P)[:, None]
    qq = np.arange(QW)[None, :]
    tri = np.concatenate(
        [np.where(kk <= qq, 0.0, NEG), np.where(kk + P <= qq, 0.0, NEG)], axis=1
    ).astype(np.float32)
    bflags = [np.full((P, 1), NEG, np.float32), np.zeros((P, 1), np.float32)]

    in_maps = []
    for core in range(2 * B):
        b, p = core // 2, core % 2
        in_maps.append(
            {"xT": xT[b][p], "wqk": wqk, "wv": wv, "mask": tri,
             "bflag": bflags[p]}
        )
    return in_maps


def _assemble(results):
    out = np.empty((B, T, HS), dtype=np.float32)
    for core in range(2 * B):
        b, p = core // 2, core % 2
        oc = results[core]["out"].astype(np.float64)  # [NCH, 65, 256]
        o = (oc[:, 0:HS, :] / oc[:, HS : HS + 1, :]).transpose(0, 2, 1)
        for g in range(NCH):
            out[b, g * CH + QW * p : g * CH + QW * p + QW, :] = o[g]
    return out


def run(x, Wk, Wq, Wv, trace=False):
    nc = _get_program()
    in_maps = _host_inputs(x, Wk, Wq, Wv)
    res = run_bass_kernel_spmd(nc, in_maps, list(range(2 * B)), trace=trace)
    return _assemble(res.results), res


def kernel(x, Wk, Wq, Wv):
    out, _ = run(x, Wk, Wq, Wv)
    return out
